# revision 15
# baseline (speedup 1.0000x reference)
"""Multi-head self-attention (b=4, n=2048, f=1024, h=16) on 8 trn2 NeuronCores.

Sharding: core c -> batch c//2, head-half c%2 (8 heads of 64 dims each).
Each core computes its 8 heads' attention and a partial output projection
(attn_slice @ Wo_rows); host sums the two partials per batch and adds bo.

Mask algebra (additive mask m_i*m_j, m in {0,1}):
  e^{s + m_i m_j} = e^s * (ca_i applied to masked keys), ca_i = m_i e + 1-m_i.
  With tokens sorted so m=1 keys come first, accumulate per head (PSUM, 65
  rows = [v | 1]):
    A = sum_{masked keys}   e^{s/32} [v | 1]
    B = sum_{unmasked keys} e^{s/32} [v | 1]
  then  w = ca_b (.) A + B  carries the numerator in rows 0:64 and the
  selected denominator in row 64:  out_i = w[0:64, i] / w[64, i].
  The two boundary 128-key chunks (7, 8) get host-masked stationaries
  S_bm = mb (.) [v|1] -> A and S_bu = (1-mb) (.) [v|1] -> B.

exp is split across engines: ScalarE ACTIVATE(Exp, scale=1/32) on cols
[0:SC] of each [128,1024] logit tile, a custom DVE op EXP2SQ_ANT
(p(t)^2 with deg-3 p ~ e^{t/64}) on cols [SC:1024].
"""

import sys

sys.path.insert(0, "/opt/trn_rl_repo")

import numpy as np
import ml_dtypes

import concourse.bass as bass
import concourse.bacc as bacc
import concourse.mybir as mybir
import concourse.tile as tile
from concourse import bass_utils
from concourse import dve_ops
from concourse.dve_spec import Spec, Src0, One, C0, C1, C2, lower, _has_src1
from concourse.dve_uop import DveOpSpec

BF16 = mybir.dt.bfloat16
F32 = mybir.dt.float32
NPBF16 = ml_dtypes.bfloat16

B, N, F, H, HD = 4, 2048, 1024, 16, 64
FH = 512          # features per core (8 heads)
NC_ = 8           # cores
NTOKC = N // 128  # 16 token chunks
NIBLK = N // 512  # 4 query blocks
NJ = N // 128     # 16 key chunks
NPAIR = 4         # head pairs per core
SW = 65           # stationary width [v | 1]
EXPFN = mybir.ActivationFunctionType.Exp
E_ = float(np.e)

# exp(t/32) ~= p(t)^2 with p(t) = 1 + EC0 t + EC1 t^2 + EC2 t^3 (fit [-26,26])
EC0, EC1, EC2 = 1.5632068e-02, 1.2325062e-04, 6.2187036e-07
import os
SC = int(os.environ.get("K_SC", "640"))  # cols exp'd on ScalarE (rest: DVE)
USE_GPSIMD = os.environ.get("K_GPSIMD", "1") == "1"
DRAIN_K = int(os.environ.get("K_DRAIN", "10"))   # o-proj ops drained per unit
WARM_K = int(os.environ.get("K_WARM", "8"))      # tail warm filler matmuls


def _register_exp_op():
    name = "EXP2SQ_ANT"
    for op in dve_ops.OPS:
        if op.name == name:
            return op
    t2 = Src0 * Src0
    p = (One + Src0 * C0) + t2 * (C1 + Src0 * C2)
    spec = Spec(
        body=p * p,
        reference=lambda in0, in1, s0, s1, imm2: (
            (1.0 + s0 * in0 + in0 * in0 * (s1 + imm2 * in0)) ** 2
        ),
    )
    row = max(dve_ops._SUB_OPCODE_FOR_NAME.values()) + 1
    assert row < 0x20
    dve_ops._SUB_OPCODE_FOR_NAME[name] = row
    shas = {}
    for ver in ("v3", "v4"):
        s = DveOpSpec(
            name=name, opcode=row, uops=lower(spec, ver=ver),
            rd1_en=_has_src1(spec),
        )
        shas[ver] = s.sha(ver)
    op = dve_ops.DveOp(name, spec, subdim=False, uops_sha=shas)
    dve_ops.OPS.append(op)
    dve_ops.CUSTOM_DVE_SPECS[name] = spec
    return op


EXP2SQ = _register_exp_op()


def _dve_exp(nc, out, in_):
    nc.vector._custom_dve(EXP2SQ, out=out, in0=in_, s0=EC0, s1=EC1, imm2=EC2)


def _emit(nc, tc, d, generic):
    """Emit the per-core program under TileContext tc."""
    consts = tc.alloc_tile_pool(name="consts", bufs=1)
    persist = tc.alloc_tile_pool(name="persist", bufs=1)

    if generic:
        # every chunk host-masked: works for any mask, no sorting assumption
        routes = [(j, "b", j, 0) for j in range(NJ)] + [
            (j, "b", NJ + j, 1) for j in range(NJ)
        ]
        bchunk = [j % NJ for j in range(2 * NJ)]   # s1b tile k -> key chunk
    else:
        # (chunk j, source, s1b tile index, acc: 0=A masked / 1=B unmasked)
        routes = (
            [(j, "s", 0, 0) for j in range(7)]
            + [(7, "b", 0, 0), (8, "b", 1, 0), (7, "b", 2, 1), (8, "b", 3, 1)]
            + [(j, "s", 0, 1) for j in range(9, NJ)]
        )
        bchunk = [7, 8, 7, 8]
    by_j = {}
    for j, src, k, acc in routes:
        by_j.setdefault(j, []).append((src, k, acc))
    nmm = {0: sum(1 for r in routes if r[3] == 0)}
    nmm[1] = len(routes) - nmm[0]
    nb_tiles = len(bchunk)

    # ---- persistent activations ----------------------------------------
    qT_sb = persist.tile([128, 4 * N], BF16)   # [feat, tok], chunk fc at cols fc*N
    kT_sb = persist.tile([128, 4 * N], BF16)
    s1_sb = persist.tile([128, NJ * 8 * SW], BF16)   # per (jc, head): [v | 1]
    s1b_sb = persist.tile([128, nb_tiles * 8 * SW], BF16)  # boundary-masked
    attnT = persist.tile([128, 4 * N], BF16)   # normalized attn, [feat, tok]

    # ================= phase 1: projections ==============================
    with tc.tile_pool(name="p1sb", bufs=1) as p1sb, \
         tc.tile_pool(name="pkt", bufs=1, space="PSUM") as pkt:
        # xT + Wk loads first (kT matmuls consume them chunk by chunk)
        xT_sb = p1sb.tile([128, 8 * N], BF16)
        wk_sb = p1sb.tile([128, 8 * FH], BF16)
        bqk = consts.tile([128, 8], F32)       # bq chunks (0-3), bk chunks (4-7)
        nc.sync.dma_start(out=bqk, in_=d["bqk"])
        for fc in range(8):
            nc.sync.dma_start(
                out=wk_sb[:, fc * FH:(fc + 1) * FH],
                in_=d["wk"][fc * 128:(fc + 1) * 128, :],
            )
            nc.sync.dma_start(
                out=xT_sb[:, fc * N:(fc + 1) * N],
                in_=d["xT"][fc * 128:(fc + 1) * 128, :],
            )

        # kT: fc-outer over 4 concurrent psum tiles (full PSUM) so the first
        # matmuls start as soon as chunk 0 of xT/Wk lands.
        for grp in range(2):
            pks = [
                pkt.tile([128, 1024], F32, tag=f"pp{t}", name=f"pk{t}")
                for t in range(4)
            ]
            for fc in range(8):
                for t in range(4):
                    fhc, half = grp * 2 + t // 2, t % 2
                    lhsT = wk_sb[:, fc * FH + fhc * 128: fc * FH + (fhc + 1) * 128]
                    for nn in range(2):
                        off = half * 1024 + nn * 512
                        nc.tensor.matmul(
                            pks[t][:, nn * 512:(nn + 1) * 512],
                            lhsT,
                            xT_sb[:, fc * N + off: fc * N + off + 512],
                            start=(fc == 0),
                            stop=(fc == 7),
                        )
            for t in range(4):
                fhc, half = grp * 2 + t // 2, t % 2
                nc.vector.tensor_scalar_add(
                    out=kT_sb[:, fhc * N + half * 1024: fhc * N + half * 1024 + 1024],
                    in0=pks[t][:],
                    scalar1=bqk[:, 4 + fhc: 5 + fhc],
                )

        # remaining loads (emitted after kT matmuls so they queue behind)
        bmask = consts.tile([128, nb_tiles], F32)
        nc.sync.dma_start(out=bmask, in_=d["bmask"])
        cab = consts.tile([SW, 4 * 512], F32)   # ca broadcast over 65 partitions
        nc.sync.dma_start(out=cab, in_=d["cab"])
        bvb = consts.tile([128, FH], F32)      # bv broadcast over partitions
        nc.sync.dma_start(out=bvb, in_=d["bvb"])
        wo_sb = consts.tile([128, 4 * 1024], BF16)
        for fc in range(4):
            nc.sync.dma_start(
                out=wo_sb[:, fc * 1024:(fc + 1) * 1024],
                in_=d["wo"][fc * 128:(fc + 1) * 128, :],
            )
        w_sb = {}
        for wname in ("wq", "wv"):
            t = p1sb.tile([128, 8 * FH], BF16, tag=wname)
            for fc in range(8):
                nc.sync.dma_start(
                    out=t[:, fc * FH:(fc + 1) * FH],
                    in_=d[wname][fc * 128:(fc + 1) * 128, :],
                )
            w_sb[wname] = t

        # qT (xT is resident by now; fc-inner keeps PSUM small)
        for fhc in range(4):
            for half in range(2):
                pk = pkt.tile(
                    [128, 1024], F32, tag=f"pp{(fhc * 2 + half) % 2}", name="pkq"
                )
                for fc in range(8):
                    lhsT = w_sb["wq"][:, fc * FH + fhc * 128: fc * FH + (fhc + 1) * 128]
                    for nn in range(2):
                        off = half * 1024 + nn * 512
                        nc.tensor.matmul(
                            pk[:, nn * 512:(nn + 1) * 512],
                            lhsT,
                            xT_sb[:, fc * N + off: fc * N + off + 512],
                            start=(fc == 0),
                            stop=(fc == 7),
                        )
                nc.vector.tensor_scalar_add(
                    out=qT_sb[:, fhc * N + half * 1024: fhc * N + half * 1024 + 1024],
                    in0=pk[:],
                    scalar1=bqk[:, fhc: fhc + 1],
                )

        # v: out[tok_chunk 128, fh 512] = xT_chunk^T @ Wv ; then build S1
        for tokc in range(NTOKC):
            pv = pkt.tile([128, FH], F32, tag=f"pp{2 + tokc % 2}", name="pv")
            for fc in range(8):
                nc.tensor.matmul(
                    pv[:],
                    xT_sb[:, fc * N + tokc * 128: fc * N + (tokc + 1) * 128],
                    w_sb["wv"][:, fc * FH:(fc + 1) * FH],
                    start=(fc == 0),
                    stop=(fc == 7),
                )
            base = tokc * 8 * SW
            s1_v = s1_sb[:, base:base + 8 * SW].rearrange("p (h c) -> p h c", h=8)
            pv_v = pv[:].rearrange("p (h c) -> p h c", h=8)
            bv_v = bvb[:].rearrange("p (h c) -> p h c", h=8)
            nc.vector.tensor_add(out=s1_v[:, :, 0:64], in0=pv_v, in1=bv_v)
            nc.vector.memset(s1_v[:, :, 64:65], 1.0)

        # boundary stationaries: S_b[k] = bmask[:, k] (.) S1[chunk bchunk[k]]
        for k in range(nb_tiles):
            jc = bchunk[k]
            nc.vector.tensor_scalar_mul(
                out=s1b_sb[:, k * 8 * SW:(k + 1) * 8 * SW],
                in0=s1_sb[:, jc * 8 * SW:(jc + 1) * 8 * SW],
                scalar1=bmask[:, k:k + 1],
            )

    # ================= phase 2: attention ================================
    with tc.tile_pool(name="pP", bufs=2, space="PSUM") as pP, \
         tc.tile_pool(name="pacc", bufs=1, space="PSUM") as pacc, \
         tc.tile_pool(name="sexp", bufs=4) as sexp, \
         tc.tile_pool(name="episb", bufs=2) as episb, \
         tc.tile_pool(name="osb", bufs=3) as osb, \
         tc.tile_pool(name="epidr", bufs=2, space="DRAM") as epidr:

        # O-projection micro-ops (one instruction each), drained into the
        # exp-pipeline warmup slack at the start of each (pair, iblk) unit.
        pending = []

        def o_ops_for_iblk(ib, fin_act=False):
            ops = []
            if os.environ.get("K_RELOADWO") == "1":
                def reload():
                    for fc in range(4):
                        nc.sync.dma_start(
                            out=wo_sb[:, fc * 1024:(fc + 1) * 1024],
                            in_=d["wo"][fc * 128:(fc + 1) * 128, :],
                        )
                ops.append(reload)
            for tokc in range(ib * 4, ib * 4 + 4):
                for half in range(2):
                    st = {}
                    if os.environ.get("K_POTAG") == "b":
                        tag = f"acc{(tokc * 2 + half) % 2 * 2 + 1}"  # acc1/acc3
                    else:
                        tag = f"acc{(tokc * 2 + half) % 4}"
                    for fc in range(4):
                        def mm(fc=fc, tokc=tokc, half=half, st=st, tag=tag):
                            if fc == 0:
                                st["po"] = pacc.tile(
                                    [128, 512], F32, tag=tag, name="po"
                                )
                            nc.tensor.matmul(
                                st["po"][:],
                                attnT[:, fc * N + tokc * 128: fc * N + (tokc + 1) * 128],
                                wo_sb[:, fc * 1024 + half * 512: fc * 1024 + half * 512 + 512],
                                start=(fc == 0),
                                stop=(fc == 3),
                            )
                        ops.append(mm)

                    def fin(tokc=tokc, half=half, st=st):
                        ot = osb.tile([128, 512], F32, tag="ot", name="ot")
                        if fin_act or (tokc + half) % 2 == 0:
                            nc.scalar.activation(
                                out=ot, in_=st["po"][:],
                                func=mybir.ActivationFunctionType.Copy,
                            )
                        else:
                            nc.vector.tensor_copy(out=ot, in_=st["po"][:])
                        nc.sync.dma_start(
                            out=d["y"][tokc * 128:(tokc + 1) * 128,
                                       half * 512:(half + 1) * 512],
                            in_=ot,
                        )
                    ops.append(fin)
            return ops

        def drain(kk):
            for _ in range(kk):
                if pending:
                    pending.pop(0)()

        for iblk in range(NIBLK):
            for pair in range(NPAIR):
                drain(DRAIN_K)
                accs = {
                    (h, a): pacc.tile(
                        [SW, 512], F32, tag=f"acc{h * 2 + a}", name=f"acc{h}{a}"
                    )
                    for h in range(2) for a in range(2)
                }
                started = {}
                left = {(h, a): nmm[a] for h in range(2) for a in range(2)}

                def qk(j):
                    P = pP.tile([128, 1024], F32, tag="logits")
                    for hl, tp in ((0, 0), (1, 64)):
                        nc.tensor.matmul(
                            P[:, hl * 512:(hl + 1) * 512],
                            kT_sb[tp:tp + 64, pair * N + j * 128: pair * N + (j + 1) * 128],
                            qT_sb[tp:tp + 64, pair * N + iblk * 512: pair * N + (iblk + 1) * 512],
                            start=True,
                            stop=True,
                            tile_position=(tp, 0),
                        )
                    return P

                P0 = qk(0)
                P1 = qk(1)
                Ptil = {0: P0, 1: P1}
                for j in range(NJ):
                    S = sexp.tile([128, 1024], BF16, tag="etil")
                    Pj = Ptil.pop(j)
                    nc.scalar.activation(
                        out=S[:, 0:SC], in_=Pj[:, 0:SC], func=EXPFN,
                        scale=1.0 / 32.0,
                    )
                    if SC < 1024:
                        _dve_exp(nc, S[:, SC:1024], Pj[:, SC:1024])
                    if j + 2 < NJ:
                        Ptil[j + 2] = qk(j + 2)
                    for src, k, a in by_j.get(j, ()):
                        buf = s1_sb if src == "s" else s1b_sb
                        idx = j if src == "s" else k
                        for hl in range(2):
                            hcore = 2 * pair + hl
                            soff = idx * 8 * SW + hcore * SW
                            acc = accs[(hl, a)]
                            nc.tensor.matmul(
                                acc[:], buf[:, soff:soff + SW],
                                S[:, hl * 512:(hl + 1) * 512],
                                start=not started.get((hl, a), False),
                                stop=left[(hl, a)] == 1,
                            )
                            started[(hl, a)] = True
                            left[(hl, a)] -= 1

                # ---- epilogue: w = ca_b (.) A + B ; out = w[0:64]/w[64] --
                ib = iblk * 512
                dstc = pair * N + iblk * 512
                stg = epidr.tile([2, 512], F32, tag="stg")
                ws = {}
                for hl in range(2):
                    u = episb.tile([SW, 512], F32, tag=f"epu{hl}")
                    nc.vector.tensor_mul(
                        out=u, in0=accs[(hl, 0)][:], in1=cab[:, ib:ib + 512]
                    )
                    w = episb.tile([SW, 512], F32, tag=f"epw{hl}")
                    nc.vector.tensor_add(out=w, in0=u, in1=accs[(hl, 1)][:])
                    rd = episb.tile([SW, 512], F32, tag=f"eprd{hl}")
                    # custom-DVE ops mis-address on a non-zero partition base;
                    # run on the full tile (same cost: column-bound)
                    nc.vector.reciprocal_approx_fast(out=rd[:], in_=w[:])
                    nc.sync.dma_start(out=stg[hl:hl + 1, :], in_=rd[64:65, :])
                    ws[hl] = w
                rball = episb.tile([64, 2 * 512], F32, tag="rball")
                nc.sync.dma_start(
                    out=rball[:],
                    in_=bass.AP(tensor=stg.tensor, offset=stg.offset,
                                ap=[[0, 64], [512, 2], [1, 512]]),
                )
                eng = nc.gpsimd if USE_GPSIMD else nc.vector
                eng.tensor_mul(
                    out=attnT[0:64, dstc:dstc + 512],
                    in0=ws[0][0:64, :], in1=rball[:, 0:512],
                )
                t3 = episb.tile([64, 512], BF16, tag="ept3")
                eng.tensor_mul(
                    out=t3, in0=ws[1][0:64, :], in1=rball[:, 512:1024]
                )
                nc.sync.dma_start(out=attnT[64:128, dstc:dstc + 512], in_=t3)

            pending.extend(o_ops_for_iblk(iblk, fin_act=(iblk == NIBLK - 1)))

        # ===== tail: drain remaining O-projection ops ======================
        warm = pacc.tile([128, 512], F32, tag="acc1", name="warm")
        for _ in range(WARM_K):
            nc.tensor.matmul(
                warm[:], wo_sb[:, 0:128], wo_sb[:, 0:512],
                start=True, stop=True,
            )
        while pending:
            pending.pop(0)()

        if "dbg" in d:
            nc.sync.dma_start(out=d["dbg"], in_=wo_sb[:])
            nc.sync.dma_start(out=d["dbg2"], in_=attnT[:])

    persist.release()
    consts.release()


_CACHE = {}


def build_program(variant="fast"):
    if variant in _CACHE:
        return _CACHE[variant]
    nc = bacc.Bacc("TRN2", target_bir_lowering=False, debug=False)
    nb_tiles = 2 * NJ if variant == "generic" else 4
    d = {}
    d["xT"] = nc.dram_tensor("xT", (F, N), BF16, kind="ExternalInput").ap()
    d["wq"] = nc.dram_tensor("wq", (F, FH), BF16, kind="ExternalInput").ap()
    d["wk"] = nc.dram_tensor("wk", (F, FH), BF16, kind="ExternalInput").ap()
    d["wv"] = nc.dram_tensor("wv", (F, FH), BF16, kind="ExternalInput").ap()
    d["wo"] = nc.dram_tensor("wo", (FH, F), BF16, kind="ExternalInput").ap()
    d["bqk"] = nc.dram_tensor("bqk", (128, 8), F32, kind="ExternalInput").ap()
    d["bvb"] = nc.dram_tensor("bvb", (128, FH), F32, kind="ExternalInput").ap()
    d["bmask"] = nc.dram_tensor(
        "bmask", (128, nb_tiles), F32, kind="ExternalInput").ap()
    d["cab"] = nc.dram_tensor("cab", (SW, 4 * 512), F32, kind="ExternalInput").ap()
    d["y"] = nc.dram_tensor("y", (N, F), F32, kind="ExternalOutput").ap()
    if os.environ.get("K_DBG") == "1":
        d["dbg"] = nc.dram_tensor(
            "dbg", (128, 4096), BF16, kind="ExternalOutput").ap()
        d["dbg2"] = nc.dram_tensor(
            "dbg2", (128, 8192), BF16, kind="ExternalOutput").ap()
    with tile.TileContext(nc) as tc:
        _emit(nc, tc, d, generic=(variant == "generic"))
    nc.compile()
    _CACHE[variant] = nc
    return nc


def make_in_maps(x, inputs_mask, Wq, bq, Wk, bk, Wv, bv, Wo, bo, variant="fast"):
    """Host-side shard prep. All args np.float32/int32 full tensors."""
    in_maps = []
    m_all = inputs_mask.astype(np.float32)
    perms = []
    for b in range(B):
        if variant == "fast":
            perms.append(np.argsort(-m_all[b], kind="stable"))
        else:
            perms.append(np.arange(N))
    for c in range(NC_):
        b, hh = c // 2, c % 2
        cs = slice(hh * FH, (hh + 1) * FH)
        m = m_all[b][perms[b]]
        xb = x[b][perms[b]]
        if variant == "fast":
            bm = np.stack([m[7 * 128:8 * 128], m[8 * 128:9 * 128]], axis=1)
            bmask = np.concatenate([bm, 1.0 - bm], axis=1)     # [128, 4]
        else:
            bm = m.reshape(NJ, 128).T                          # [128, NJ]
            bmask = np.concatenate([bm, 1.0 - bm], axis=1)     # [128, 2NJ]
        ca = m * E_ + (1.0 - m)
        im = {
            "xT": np.ascontiguousarray(xb.T).astype(NPBF16),
            "wq": Wq[:, cs].astype(NPBF16),
            "wk": Wk[:, cs].astype(NPBF16),
            "wv": Wv[:, cs].astype(NPBF16),
            "wo": np.ascontiguousarray(Wo[cs, :]).astype(NPBF16),
            "bqk": np.stack(
                [bq[cs].reshape(4, 128), bk[cs].reshape(4, 128)], axis=0
            ).reshape(8, 128).T.astype(np.float32).copy(),
            "bvb": np.broadcast_to(bv[cs], (128, FH)).astype(np.float32).copy(),
            "bmask": bmask.astype(np.float32).copy(),
            "cab": np.broadcast_to(ca, (SW, N)).astype(np.float32).copy(),
        }
        in_maps.append(im)
    return in_maps, perms


def kernel(x, inputs_mask, Wq, bq, Wk, bk, Wv, bv, Wo, bo):
    x = np.asarray(x, dtype=np.float32)
    inputs_mask = np.asarray(inputs_mask)
    Wq, bq = np.asarray(Wq, np.float32), np.asarray(bq, np.float32)
    Wk, bk = np.asarray(Wk, np.float32), np.asarray(bk, np.float32)
    Wv, bv = np.asarray(Wv, np.float32), np.asarray(bv, np.float32)
    Wo, bo = np.asarray(Wo, np.float32), np.asarray(bo, np.float32)

    # fast path: after sorting, the mask boundary must land in key chunks 7/8
    c1 = inputs_mask.astype(np.int64).sum(axis=1)
    variant = "fast" if bool(np.all((c1 >= 897) & (c1 <= 1152))) else "generic"
    nc = build_program(variant)
    in_maps, perms = make_in_maps(
        x, inputs_mask, Wq, bq, Wk, bk, Wv, bv, Wo, bo, variant=variant
    )
    res = bass_utils.run_bass_kernel_spmd(nc, in_maps, core_ids=list(range(NC_)))
    out = np.empty((B, N, F), dtype=np.float32)
    for b in range(B):
        out[b][perms[b]] = (
            res.results[2 * b]["y"] + res.results[2 * b + 1]["y"] + bo
        )
    return out


# revision 21
# speedup vs baseline: 1.0917x; 1.0917x over previous
"""Multi-head self-attention (b=4, n=2048, f=1024, h=16) on 8 trn2 NeuronCores.

Sharding: core c -> batch c//2, head-half c%2 (8 heads of 64 dims each).
Each core computes its 8 heads' attention and a partial output projection
(attn_slice @ Wo_rows); host sums the two partials per batch and adds bo.

Mask algebra (additive mask m_i*m_j, m in {0,1}):
  e^{s + m_i m_j} = e^s * (ca_i applied to masked keys), ca_i = m_i e + 1-m_i.
  With tokens sorted so m=1 keys come first, accumulate per head (PSUM, 65
  rows = [v | 1]):
    A = sum_{masked keys}   e^{s/32} [v | 1]
    B = sum_{unmasked keys} e^{s/32} [v | 1]
  then  w = ca_b (.) A + B  carries the numerator in rows 0:64 and the
  selected denominator in row 64:  out_i = w[0:64, i] / w[64, i].
  The two boundary 128-key chunks (7, 8) get host-masked stationaries
  S_bm = mb (.) [v|1] -> A and S_bu = (1-mb) (.) [v|1] -> B.

exp is split across engines: ScalarE ACTIVATE(Exp, scale=1/32) on cols
[0:SC] of each [128,1024] logit tile, a custom DVE op EXP2SQ_ANT
(p(t)^2 with deg-3 p ~ e^{t/64}) on cols [SC:1024].
"""

import sys

sys.path.insert(0, "/opt/trn_rl_repo")

import numpy as np
import ml_dtypes

import concourse.bass as bass
import concourse.bacc as bacc
import concourse.mybir as mybir
import concourse.tile as tile
from concourse import bass_utils
from concourse import dve_ops
from concourse.dve_spec import Spec, Src0, One, C0, C1, C2, lower, _has_src1
from concourse.dve_uop import DveOpSpec

BF16 = mybir.dt.bfloat16
F32 = mybir.dt.float32
NPBF16 = ml_dtypes.bfloat16

B, N, F, H, HD = 4, 2048, 1024, 16, 64
FH = 512          # features per core (8 heads)
NC_ = 8           # cores
NTOKC = N // 128  # 16 token chunks
NIBLK = N // 512  # 4 query blocks
NJ = N // 128     # 16 key chunks
NPAIR = 4         # head pairs per core
SW = 65           # stationary width [v | 1]
EXPFN = mybir.ActivationFunctionType.Exp
E_ = float(np.e)

# exp(t/32) ~= p(t)^2 with p(t) = 1 + EC0 t + EC1 t^2 + EC2 t^3 (fit [-26,26])
EC0, EC1, EC2 = 1.5632068e-02, 1.2325062e-04, 6.2187036e-07
import os
# exp col split must be the PSUM bank boundary (512): ScalarE reads bank 0,
# DVE reads bank 1 of each logit tile -- parallel PSUM access needs
# different banks.
SC = int(os.environ.get("K_SC", "512"))
USE_GPSIMD = os.environ.get("K_GPSIMD", "1") == "1"
DRAIN_K = int(os.environ.get("K_DRAIN", "10"))   # o-proj ops drained per unit
WARM_K = int(os.environ.get("K_WARM", "8"))      # tail warm filler matmuls


def _register_exp_op():
    name = "EXP2SQ_ANT"
    for op in dve_ops.OPS:
        if op.name == name:
            return op
    t2 = Src0 * Src0
    p = (One + Src0 * C0) + t2 * (C1 + Src0 * C2)
    spec = Spec(
        body=p * p,
        reference=lambda in0, in1, s0, s1, imm2: (
            (1.0 + s0 * in0 + in0 * in0 * (s1 + imm2 * in0)) ** 2
        ),
    )
    row = max(dve_ops._SUB_OPCODE_FOR_NAME.values()) + 1
    assert row < 0x20
    dve_ops._SUB_OPCODE_FOR_NAME[name] = row
    shas = {}
    for ver in ("v3", "v4"):
        s = DveOpSpec(
            name=name, opcode=row, uops=lower(spec, ver=ver),
            rd1_en=_has_src1(spec),
        )
        shas[ver] = s.sha(ver)
    op = dve_ops.DveOp(name, spec, subdim=False, uops_sha=shas)
    dve_ops.OPS.append(op)
    dve_ops.CUSTOM_DVE_SPECS[name] = spec
    return op


EXP2SQ = _register_exp_op()


def _dve_exp(nc, out, in_):
    nc.vector._custom_dve(EXP2SQ, out=out, in0=in_, s0=EC0, s1=EC1, imm2=EC2)


def _emit(nc, tc, d, generic):
    """Emit the per-core program under TileContext tc."""
    consts = tc.alloc_tile_pool(name="consts", bufs=1)
    persist = tc.alloc_tile_pool(name="persist", bufs=1)

    if generic:
        # every chunk host-masked: works for any mask, no sorting assumption
        routes = [(j, "b", j, 0) for j in range(NJ)] + [
            (j, "b", NJ + j, 1) for j in range(NJ)
        ]
        bchunk = [j % NJ for j in range(2 * NJ)]   # s1b tile k -> key chunk
    else:
        # (chunk j, source, s1b tile index, acc: 0=A masked / 1=B unmasked)
        routes = (
            [(j, "s", 0, 0) for j in range(7)]
            + [(7, "b", 0, 0), (8, "b", 1, 0), (7, "b", 2, 1), (8, "b", 3, 1)]
            + [(j, "s", 0, 1) for j in range(9, NJ)]
        )
        bchunk = [7, 8, 7, 8]
    by_j = {}
    for j, src, k, acc in routes:
        by_j.setdefault(j, []).append((src, k, acc))
    nmm = {0: sum(1 for r in routes if r[3] == 0)}
    nmm[1] = len(routes) - nmm[0]
    nb_tiles = len(bchunk)

    # ---- persistent activations ----------------------------------------
    qT_sb = persist.tile([128, 4 * N], BF16)   # [feat, tok], chunk fc at cols fc*N
    kT_sb = persist.tile([128, 4 * N], BF16)
    s1_sb = persist.tile([128, NJ * 8 * SW], BF16)   # per (jc, head): [v | 1]
    s1b_sb = persist.tile([128, nb_tiles * 8 * SW], BF16)  # boundary-masked
    attnT = persist.tile([128, 4 * N], BF16)   # normalized attn, [feat, tok]

    # ================= phase 1: projections ==============================
    with tc.tile_pool(name="p1sb", bufs=1) as p1sb, \
         tc.tile_pool(name="pkt", bufs=1, space="PSUM") as pkt:
        # xT + Wk loads first (kT matmuls consume them chunk by chunk)
        xT_sb = p1sb.tile([128, 8 * N], BF16)
        wk_sb = p1sb.tile([128, 8 * FH], BF16)
        bqk = consts.tile([128, 8], F32)       # bq chunks (0-3), bk chunks (4-7)
        nc.sync.dma_start(out=bqk, in_=d["bqk"])
        for fc in range(8):
            nc.sync.dma_start(
                out=wk_sb[:, fc * FH:(fc + 1) * FH],
                in_=d["wk"][fc * 128:(fc + 1) * 128, :],
            )
            nc.sync.dma_start(
                out=xT_sb[:, fc * N:(fc + 1) * N],
                in_=d["xT"][fc * 128:(fc + 1) * 128, :],
            )

        # kT: fc-outer over 4 concurrent psum tiles (full PSUM) so the first
        # matmuls start as soon as chunk 0 of xT/Wk lands.
        for grp in range(2):
            pks = [
                pkt.tile([128, 1024], F32, tag=f"pp{t}", name=f"pk{t}")
                for t in range(4)
            ]
            for fc in range(8):
                for t in range(4):
                    fhc, half = grp * 2 + t // 2, t % 2
                    lhsT = wk_sb[:, fc * FH + fhc * 128: fc * FH + (fhc + 1) * 128]
                    for nn in range(2):
                        off = half * 1024 + nn * 512
                        nc.tensor.matmul(
                            pks[t][:, nn * 512:(nn + 1) * 512],
                            lhsT,
                            xT_sb[:, fc * N + off: fc * N + off + 512],
                            start=(fc == 0),
                            stop=(fc == 7),
                        )
            for t in range(4):
                fhc, half = grp * 2 + t // 2, t % 2
                nc.vector.tensor_scalar_add(
                    out=kT_sb[:, fhc * N + half * 1024: fhc * N + half * 1024 + 1024],
                    in0=pks[t][:],
                    scalar1=bqk[:, 4 + fhc: 5 + fhc],
                )

        # remaining loads (emitted after kT matmuls so they queue behind)
        bmask = consts.tile([128, nb_tiles], F32)
        nc.sync.dma_start(out=bmask, in_=d["bmask"])
        cab = consts.tile([SW, 4 * 512], F32)   # ca broadcast over 65 partitions
        nc.sync.dma_start(out=cab, in_=d["cab"])
        bvb = consts.tile([128, FH], F32)      # bv broadcast over partitions
        nc.sync.dma_start(out=bvb, in_=d["bvb"])
        wo_sb = consts.tile([128, 4 * 1024], BF16)
        for fc in range(4):
            nc.sync.dma_start(
                out=wo_sb[:, fc * 1024:(fc + 1) * 1024],
                in_=d["wo"][fc * 128:(fc + 1) * 128, :],
            )
        w_sb = {}
        for wname in ("wq", "wv"):
            t = p1sb.tile([128, 8 * FH], BF16, tag=wname)
            for fc in range(8):
                nc.sync.dma_start(
                    out=t[:, fc * FH:(fc + 1) * FH],
                    in_=d[wname][fc * 128:(fc + 1) * 128, :],
                )
            w_sb[wname] = t

        # qT (xT is resident by now; fc-inner keeps PSUM small)
        for fhc in range(4):
            for half in range(2):
                pk = pkt.tile(
                    [128, 1024], F32, tag=f"pp{(fhc * 2 + half) % 2}", name="pkq"
                )
                for fc in range(8):
                    lhsT = w_sb["wq"][:, fc * FH + fhc * 128: fc * FH + (fhc + 1) * 128]
                    for nn in range(2):
                        off = half * 1024 + nn * 512
                        nc.tensor.matmul(
                            pk[:, nn * 512:(nn + 1) * 512],
                            lhsT,
                            xT_sb[:, fc * N + off: fc * N + off + 512],
                            start=(fc == 0),
                            stop=(fc == 7),
                        )
                nc.vector.tensor_scalar_add(
                    out=qT_sb[:, fhc * N + half * 1024: fhc * N + half * 1024 + 1024],
                    in0=pk[:],
                    scalar1=bqk[:, fhc: fhc + 1],
                )

        # v: out[tok_chunk 128, fh 512] = xT_chunk^T @ Wv ; then build S1
        for tokc in range(NTOKC):
            pv = pkt.tile([128, FH], F32, tag=f"pp{2 + tokc % 2}", name="pv")
            for fc in range(8):
                nc.tensor.matmul(
                    pv[:],
                    xT_sb[:, fc * N + tokc * 128: fc * N + (tokc + 1) * 128],
                    w_sb["wv"][:, fc * FH:(fc + 1) * FH],
                    start=(fc == 0),
                    stop=(fc == 7),
                )
            base = tokc * 8 * SW
            s1_v = s1_sb[:, base:base + 8 * SW].rearrange("p (h c) -> p h c", h=8)
            pv_v = pv[:].rearrange("p (h c) -> p h c", h=8)
            bv_v = bvb[:].rearrange("p (h c) -> p h c", h=8)
            nc.vector.tensor_add(out=s1_v[:, :, 0:64], in0=pv_v, in1=bv_v)
            nc.vector.memset(s1_v[:, :, 64:65], 1.0)

        # boundary stationaries: S_b[k] = bmask[:, k] (.) S1[chunk bchunk[k]]
        for k in range(nb_tiles):
            jc = bchunk[k]
            nc.vector.tensor_scalar_mul(
                out=s1b_sb[:, k * 8 * SW:(k + 1) * 8 * SW],
                in0=s1_sb[:, jc * 8 * SW:(jc + 1) * 8 * SW],
                scalar1=bmask[:, k:k + 1],
            )

    # ================= phase 2: attention ================================
    with tc.tile_pool(name="pP", bufs=2, space="PSUM") as pP, \
         tc.tile_pool(name="pacc", bufs=1, space="PSUM") as pacc, \
         tc.tile_pool(name="sexp", bufs=4) as sexp, \
         tc.tile_pool(name="episb", bufs=2) as episb, \
         tc.tile_pool(name="osb", bufs=3) as osb, \
         tc.tile_pool(name="epidr", bufs=2, space="DRAM") as epidr:

        # O-projection micro-ops (one instruction each), drained into the
        # exp-pipeline warmup slack at the start of each (pair, iblk) unit.
        pending = []

        def o_ops_for_iblk(ib, fin_act=False):
            ops = []
            for tokc in range(ib * 4, ib * 4 + 4):
                for half in range(2):
                    st = {}
                    tag = f"acc{(tokc * 2 + half) % 4}"
                    for fc in range(4):
                        def mm(fc=fc, tokc=tokc, half=half, st=st, tag=tag):
                            if fc == 0:
                                st["po"] = pacc.tile(
                                    [128, 512], F32, tag=tag, name="po"
                                )
                            nc.tensor.matmul(
                                st["po"][:],
                                attnT[:, fc * N + tokc * 128: fc * N + (tokc + 1) * 128],
                                wo_sb[:, fc * 1024 + half * 512: fc * 1024 + half * 512 + 512],
                                start=(fc == 0),
                                stop=(fc == 3),
                            )
                        ops.append(mm)

                    def fin(tokc=tokc, half=half, st=st):
                        ot = osb.tile([128, 512], F32, tag="ot", name="ot")
                        if fin_act or (tokc + half) % 2 == 0:
                            nc.scalar.activation(
                                out=ot, in_=st["po"][:],
                                func=mybir.ActivationFunctionType.Copy,
                            )
                        else:
                            nc.vector.tensor_copy(out=ot, in_=st["po"][:])
                        nc.sync.dma_start(
                            out=d["y"][tokc * 128:(tokc + 1) * 128,
                                       half * 512:(half + 1) * 512],
                            in_=ot,
                        )
                    ops.append(fin)
            return ops

        pending_epi = []

        def drain(kk):
            for _ in range(kk):
                if pending:
                    pending.pop(0)()

        def drain_epi(kk):
            for _ in range(kk):
                if pending_epi:
                    pending_epi.pop(0)()

        for iblk in range(NIBLK):
            for pair in range(NPAIR):
                first_of_iblk = pair == 0
                qk_first = []
                accs = {
                    (h, a): pacc.tile(
                        [SW, 512], F32, tag=f"acc{h * 2 + a}", name=f"acc{h}{a}"
                    )
                    for h in range(2) for a in range(2)
                }
                started = {}
                left = {(h, a): nmm[a] for h in range(2) for a in range(2)}

                def qk(j):
                    P = pP.tile([128, 1024], F32, tag="logits")
                    for hl, tp in ((0, 0), (1, 64)):
                        nc.tensor.matmul(
                            P[:, hl * 512:(hl + 1) * 512],
                            kT_sb[tp:tp + 64, pair * N + j * 128: pair * N + (j + 1) * 128],
                            qT_sb[tp:tp + 64, pair * N + iblk * 512: pair * N + (iblk + 1) * 512],
                            start=True,
                            stop=True,
                            tile_position=(tp, 0),
                        )
                    return P

                P0 = qk(0)
                P1 = qk(1)
                Ptil = {0: P0, 1: P1}
                # o-proj drains go at pair start (po tiles must precede this
                # pair's accs in tag order), after QK(0)/(1) so the exp chain
                # starts immediately
                drain(0 if first_of_iblk else 14)
                for j in range(NJ):
                    S = sexp.tile([128, 1024], BF16, tag="etil")
                    Pj = Ptil.pop(j)
                    # every 3rd j runs fully on ScalarE so the DVE has slack
                    # for the deferred epilogue ops
                    sc = 1024 if (SC < 1024 and j % 3 == 2) else SC
                    nc.scalar.activation(
                        out=S[:, 0:sc], in_=Pj[:, 0:sc], func=EXPFN,
                        scale=1.0 / 32.0,
                    )
                    if sc < 1024:
                        _dve_exp(nc, S[:, sc:1024], Pj[:, sc:1024])
                    drain_epi(1)
                    if j + 2 < NJ:
                        Ptil[j + 2] = qk(j + 2)
                    for src, k, a in by_j.get(j, ()):
                        buf = s1_sb if src == "s" else s1b_sb
                        idx = j if src == "s" else k
                        for hl in range(2):
                            hcore = 2 * pair + hl
                            soff = idx * 8 * SW + hcore * SW
                            acc = accs[(hl, a)]
                            nc.tensor.matmul(
                                acc[:], buf[:, soff:soff + SW],
                                S[:, hl * 512:(hl + 1) * 512],
                                start=not started.get((hl, a), False),
                                stop=left[(hl, a)] == 1,
                            )
                            started[(hl, a)] = True
                            left[(hl, a)] -= 1

                # ---- epilogue: w = ca_b (.) A + B ; out = w[0:64]/w[64] --
                # deferred: drained into the next pair's j-loop (1 op per
                # step) so the DVE FIFO keeps serving exps.
                ib = iblk * 512
                dstc = pair * N + iblk * 512
                st = {}

                def ep_u(hl, accs=accs, st=st, ib=ib):
                    u = episb.tile([SW, 512], F32, tag=f"epu{hl}", name="u")
                    nc.vector.tensor_mul(
                        out=u, in0=accs[(hl, 0)][:], in1=cab[:, ib:ib + 512]
                    )
                    st[f"u{hl}"] = u

                def ep_w(hl, accs=accs, st=st):
                    w = episb.tile([SW, 512], F32, tag=f"epw{hl}", name="w")
                    nc.vector.tensor_add(
                        out=w, in0=st[f"u{hl}"], in1=accs[(hl, 1)][:]
                    )
                    st[f"w{hl}"] = w
                    # stage the denominator row for the batched reciprocal
                    wr = episb.tile([2, 512], F32, tag="wr", name="wr") \
                        if hl == 0 else st["wr"]
                    st["wr"] = wr
                    nc.sync.dma_start(
                        out=wr[hl:hl + 1, :], in_=st[f"w{hl}"][64:65, :]
                    )

                def ep_rd(st=st):
                    rd = episb.tile([2, 512], F32, tag="eprd", name="rd")
                    nc.vector.reciprocal_approx_fast(out=rd[:], in_=st["wr"])
                    stg = epidr.tile([2, 512], F32, tag="stg", name="stg")
                    nc.sync.dma_start(out=stg[:], in_=rd[:])
                    rball = episb.tile([64, 2 * 512], F32, tag="rball",
                                       name="rball")
                    nc.sync.dma_start(
                        out=rball[:],
                        in_=bass.AP(tensor=stg.tensor, offset=stg.offset,
                                    ap=[[0, 64], [512, 2], [1, 512]]),
                    )
                    st["rball"] = rball

                def ep_out(hl, st=st, dstc=dstc):
                    eng = nc.gpsimd if USE_GPSIMD else nc.vector
                    if hl == 0:
                        eng.tensor_mul(
                            out=attnT[0:64, dstc:dstc + 512],
                            in0=st["w0"][0:64, :],
                            in1=st["rball"][:, 0:512],
                        )
                    else:
                        t3 = episb.tile([64, 512], BF16, tag="ept3", name="t3")
                        eng.tensor_mul(
                            out=t3, in0=st["w1"][0:64, :],
                            in1=st["rball"][:, 512:1024],
                        )
                        nc.sync.dma_start(
                            out=attnT[64:128, dstc:dstc + 512], in_=t3
                        )

                pending_epi.extend([
                    lambda: ep_u(0), lambda: ep_w(0),
                    lambda: ep_u(1), lambda: ep_w(1),
                    ep_rd, lambda: ep_out(0), lambda: ep_out(1),
                ])

            pending.extend(o_ops_for_iblk(iblk, fin_act=(iblk == NIBLK - 1)))

        drain_epi(99)

        # ===== tail: drain remaining O-projection ops ======================
        warm = pacc.tile([128, 512], F32, tag="acc1", name="warm")
        for _ in range(WARM_K):
            nc.tensor.matmul(
                warm[:], wo_sb[:, 0:128], wo_sb[:, 0:512],
                start=True, stop=True,
            )
        while pending:
            pending.pop(0)()

        if "dbg" in d:
            nc.sync.dma_start(out=d["dbg"], in_=wo_sb[:])
            nc.sync.dma_start(out=d["dbg2"], in_=attnT[:])

    persist.release()
    consts.release()


_CACHE = {}


def build_program(variant="fast"):
    if variant in _CACHE:
        return _CACHE[variant]
    nc = bacc.Bacc("TRN2", target_bir_lowering=False, debug=False)
    nb_tiles = 2 * NJ if variant == "generic" else 4
    d = {}
    d["xT"] = nc.dram_tensor("xT", (F, N), BF16, kind="ExternalInput").ap()
    d["wq"] = nc.dram_tensor("wq", (F, FH), BF16, kind="ExternalInput").ap()
    d["wk"] = nc.dram_tensor("wk", (F, FH), BF16, kind="ExternalInput").ap()
    d["wv"] = nc.dram_tensor("wv", (F, FH), BF16, kind="ExternalInput").ap()
    d["wo"] = nc.dram_tensor("wo", (FH, F), BF16, kind="ExternalInput").ap()
    d["bqk"] = nc.dram_tensor("bqk", (128, 8), F32, kind="ExternalInput").ap()
    d["bvb"] = nc.dram_tensor("bvb", (128, FH), F32, kind="ExternalInput").ap()
    d["bmask"] = nc.dram_tensor(
        "bmask", (128, nb_tiles), F32, kind="ExternalInput").ap()
    d["cab"] = nc.dram_tensor("cab", (SW, 4 * 512), F32, kind="ExternalInput").ap()
    d["y"] = nc.dram_tensor("y", (N, F), F32, kind="ExternalOutput").ap()
    if os.environ.get("K_DBG") == "1":
        d["dbg"] = nc.dram_tensor(
            "dbg", (128, 4096), BF16, kind="ExternalOutput").ap()
        d["dbg2"] = nc.dram_tensor(
            "dbg2", (128, 8192), BF16, kind="ExternalOutput").ap()
    with tile.TileContext(nc) as tc:
        _emit(nc, tc, d, generic=(variant == "generic"))
    nc.compile()
    _CACHE[variant] = nc
    return nc


def make_in_maps(x, inputs_mask, Wq, bq, Wk, bk, Wv, bv, Wo, bo, variant="fast"):
    """Host-side shard prep. All args np.float32/int32 full tensors."""
    in_maps = []
    m_all = inputs_mask.astype(np.float32)
    perms = []
    for b in range(B):
        if variant == "fast":
            perms.append(np.argsort(-m_all[b], kind="stable"))
        else:
            perms.append(np.arange(N))
    for c in range(NC_):
        b, hh = c // 2, c % 2
        cs = slice(hh * FH, (hh + 1) * FH)
        m = m_all[b][perms[b]]
        xb = x[b][perms[b]]
        if variant == "fast":
            bm = np.stack([m[7 * 128:8 * 128], m[8 * 128:9 * 128]], axis=1)
            bmask = np.concatenate([bm, 1.0 - bm], axis=1)     # [128, 4]
        else:
            bm = m.reshape(NJ, 128).T                          # [128, NJ]
            bmask = np.concatenate([bm, 1.0 - bm], axis=1)     # [128, 2NJ]
        ca = m * E_ + (1.0 - m)
        im = {
            "xT": np.ascontiguousarray(xb.T).astype(NPBF16),
            "wq": Wq[:, cs].astype(NPBF16),
            "wk": Wk[:, cs].astype(NPBF16),
            "wv": Wv[:, cs].astype(NPBF16),
            "wo": np.ascontiguousarray(Wo[cs, :]).astype(NPBF16),
            "bqk": np.stack(
                [bq[cs].reshape(4, 128), bk[cs].reshape(4, 128)], axis=0
            ).reshape(8, 128).T.astype(np.float32).copy(),
            "bvb": np.broadcast_to(bv[cs], (128, FH)).astype(np.float32).copy(),
            "bmask": bmask.astype(np.float32).copy(),
            "cab": np.broadcast_to(ca, (SW, N)).astype(np.float32).copy(),
        }
        in_maps.append(im)
    return in_maps, perms


def kernel(x, inputs_mask, Wq, bq, Wk, bk, Wv, bv, Wo, bo):
    x = np.asarray(x, dtype=np.float32)
    inputs_mask = np.asarray(inputs_mask)
    Wq, bq = np.asarray(Wq, np.float32), np.asarray(bq, np.float32)
    Wk, bk = np.asarray(Wk, np.float32), np.asarray(bk, np.float32)
    Wv, bv = np.asarray(Wv, np.float32), np.asarray(bv, np.float32)
    Wo, bo = np.asarray(Wo, np.float32), np.asarray(bo, np.float32)

    # fast path: after sorting, the mask boundary must land in key chunks 7/8
    c1 = inputs_mask.astype(np.int64).sum(axis=1)
    variant = "fast" if bool(np.all((c1 >= 897) & (c1 <= 1152))) else "generic"
    nc = build_program(variant)
    in_maps, perms = make_in_maps(
        x, inputs_mask, Wq, bq, Wk, bk, Wv, bv, Wo, bo, variant=variant
    )
    res = bass_utils.run_bass_kernel_spmd(nc, in_maps, core_ids=list(range(NC_)))
    out = np.empty((B, N, F), dtype=np.float32)
    for b in range(B):
        out[b][perms[b]] = (
            res.results[2 * b]["y"] + res.results[2 * b + 1]["y"] + bo
        )
    return out
